# revision 1
# baseline (speedup 1.0000x reference)
"""DBRX MoE experts kernel for 8 Trainium2 NeuronCores.

Strategy: expert-parallel (E=8 == n_cores). Host dispatches tokens to their
top-k experts (gather), each core runs the full FFN for one expert over its
routed tokens with fp16 matmul inputs + fp32 PSUM accumulation (fp16 has the
same 10-bit significand as TF32 -> ~5e-4 absmax-rel end-to-end), host applies
the combine weights and scatter-adds partial outputs.

Per-core dataflow (F=2048 token dim, H=4096 hidden, C = max routed tokens
rounded to 8; token blocks of width <=512, last block trimmed to fit C):
  xt   [16,128,C]   x^T tiles (k-chunk, partition=f, free=c)
  w1t  [32,128,16,128] per m-chunk lhsT tiles (partition=f_in_chunk)
  w2t  [32,128,2048] w2^T tiles (partition=h_in_chunk, free=f)
  Phase A per token block: gate^T/up^T = w1^T x^T / v1^T x^T (PSUM),
    P = silu(gate^T) * up^T stored fp16 in SBUF [4096 x W].
  Phase B: y^T[f,c] = sum_h w2t[h,f] P[h,c], PSUM accumulation over all 32
    h-chunks, f processed in groups of 4x128 chunks (4 PSUM banks).
Host combines: out[token] += cw[token,e] * y_e[token].
"""
import numpy as np

F = 2048          # token feature dim (ffn in reference)
H = 4096          # expert hidden dim
E = 8             # experts == cores
CB = 512          # token block (PSUM free width, fp32)
KC = F // 128     # 16 k-chunks
MH = H // 128     # 32 m-chunks
FG = 4            # f-chunks per phase-B group (PSUM banks used for y)

_cache = {}

PRECISION = "f16"   # "f32r" (TF32) or "f16" — both ~5e-4 absmax-rel
BUFS = {"xt": 32, "w1": 6, "v1": 6, "w2": 12, "p": MH + 16, "tmp": 6, "y": 12}


def _round_tf32(x: np.ndarray) -> np.ndarray:
    """Convert to the matmul input dtype: fp16, or TF32-rounded fp32."""
    if PRECISION == "f16":
        return np.ascontiguousarray(x).astype(np.float16)
    u = np.ascontiguousarray(x).view(np.uint32)
    bias = np.uint32(0x00000FFF) + ((u >> np.uint32(13)) & np.uint32(1))
    return ((u + bias) & np.uint32(0xFFFFE000)).view(np.float32)


def _build(C, loop_r=None):
    import contextlib
    import concourse.mybir as mybir
    import concourse.tile as tile
    from concourse import bacc

    F32 = mybir.dt.float32
    F32R = (mybir.dt.float16 if PRECISION == "f16" else mybir.dt.float32r)
    Silu = mybir.ActivationFunctionType.Silu

    widths = [CB] * (C // CB)
    if C % CB:
        widths.append(C % CB)
    nc = bacc.Bacc("TRN2", target_bir_lowering=False, debug=False)
    xt = nc.dram_tensor("xt", [KC, 128, C], F32R, kind="ExternalInput").ap()
    w1t = nc.dram_tensor("w1t", [MH, 128, KC, 128], F32R, kind="ExternalInput").ap()
    v1t = nc.dram_tensor("v1t", [MH, 128, KC, 128], F32R, kind="ExternalInput").ap()
    w2t = nc.dram_tensor("w2t", [MH, 128, F], F32R, kind="ExternalInput").ap()
    yt = nc.dram_tensor("yt", [KC, 128, C], F32, kind="ExternalOutput").ap()

    with tile.TileContext(nc) as tc:
        with tc.tile_pool(name="xtp", bufs=BUFS["xt"]) as xt_pool, \
             tc.tile_pool(name="w1p", bufs=BUFS["w1"]) as w1_pool, \
             tc.tile_pool(name="v1p", bufs=BUFS["v1"]) as v1_pool, \
             tc.tile_pool(name="w2p", bufs=BUFS["w2"]) as w2_pool, \
             tc.tile_pool(name="pp", bufs=BUFS["p"]) as p_pool, \
             tc.tile_pool(name="tmpp", bufs=BUFS["tmp"]) as tmp_pool, \
             tc.tile_pool(name="yp", bufs=BUFS["y"]) as y_pool, \
             tc.tile_pool(name="ps", bufs=8, space="PSUM") as psum, \
             (tc.For_i(0, loop_r, 1) if loop_r else contextlib.nullcontext()):
            c0 = 0
            for cb, W in enumerate(widths):
                # load x^T tiles for this token block
                xts = []
                for k in range(KC):
                    t = xt_pool.tile([128, W], F32R, tag="xt", name=f"xt{cb}_{k}")
                    nc.sync.dma_start(t[:], xt[k][:, c0:c0 + W])
                    xts.append(t)

                # Phase A: P[m] = silu(w1^T x^T) * (v1^T x^T), all m-chunks
                ptiles = []
                for m in range(MH):
                    w1m = w1_pool.tile([128, KC * 128], F32R, tag="w1")
                    nc.sync.dma_start(
                        w1m[:], w1t[m].rearrange("p k j -> p (k j)"))
                    v1m = v1_pool.tile([128, KC * 128], F32R, tag="v1")
                    nc.sync.dma_start(
                        v1m[:], v1t[m].rearrange("p k j -> p (k j)"))

                    gate = psum.tile([128, W], F32, tag="ps", name=f"gate{cb}_{m}")
                    for k in range(KC):
                        nc.tensor.matmul(
                            gate[:], w1m[:, k * 128:(k + 1) * 128], xts[k][:],
                            start=(k == 0), stop=(k == KC - 1))
                    up = psum.tile([128, W], F32, tag="ps", name=f"up{cb}_{m}")
                    for k in range(KC):
                        nc.tensor.matmul(
                            up[:], v1m[:, k * 128:(k + 1) * 128], xts[k][:],
                            start=(k == 0), stop=(k == KC - 1))

                    tmp = tmp_pool.tile([128, W], F32, tag="tmp", name=f"tmp{cb}_{m}")
                    nc.scalar.activation(tmp[:], gate[:], Silu)
                    pm = p_pool.tile([128, W], F32R, tag="p", name=f"pm{cb}_{m}")
                    nc.vector.tensor_mul(pm[:], tmp[:], up[:])
                    ptiles.append(pm)

                # Phase B: y^T[f,c] accumulated over all m, f in groups of FG
                for g in range(KC // FG):
                    ypsums = [psum.tile([128, W], F32, tag="ps",
                                        name=f"yps{cb}_{g}_{j}")
                              for j in range(FG)]
                    for m in range(MH):
                        w2m = w2_pool.tile([128, FG * 128], F32R, tag="w2")
                        nc.sync.dma_start(
                            w2m[:],
                            w2t[m][:, g * FG * 128:(g + 1) * FG * 128])
                        for j in range(FG):
                            nc.tensor.matmul(
                                ypsums[j][:],
                                w2m[:, j * 128:(j + 1) * 128],
                                ptiles[m][:],
                                start=(m == 0), stop=(m == MH - 1))
                    for j in range(FG):
                        fc = g * FG + j
                        ysb = y_pool.tile([128, W], F32, tag="y",
                                          name=f"ysb{cb}_{g}_{j}")
                        nc.vector.tensor_copy(ysb[:], ypsums[j][:])
                        nc.sync.dma_start(yt[fc][:, c0:c0 + W], ysb[:])
                c0 += W

    nc.compile()
    return nc


def _get_nc(C):
    if C not in _cache:
        _cache[C] = _build(C)
    return _cache[C]


def kernel(hidden_states, top_k_weights, w1, v1, w2, top_k_index):
    from concourse.bass_utils import run_bass_kernel_spmd

    hidden_states = np.asarray(hidden_states)
    top_k_weights = np.asarray(top_k_weights, dtype=np.float32)
    top_k_index = np.asarray(top_k_index)
    w1 = np.asarray(w1, dtype=np.float32)
    v1 = np.asarray(v1, dtype=np.float32)
    w2 = np.asarray(w2, dtype=np.float32)

    B, S, Fdim = hidden_states.shape
    assert Fdim == F
    T = B * S
    x = hidden_states.reshape(T, F).astype(np.float32)

    # host-side routing/dispatch: tokens + combined weights per expert
    sels, cws = [], []
    for e in range(E):
        hit = (top_k_index == e)
        any_hit = hit.any(axis=1)
        sel = np.nonzero(any_hit)[0]
        cw = (top_k_weights * hit).sum(axis=1)[sel].astype(np.float32)
        sels.append(sel)
        cws.append(cw)

    max_n = max(len(s) for s in sels)
    C = max(128, ((max_n + 7) // 8) * 8)
    nc = _get_nc(C)

    in_maps = []
    for e in range(E):
        sel = sels[e]
        n = len(sel)
        xe = np.zeros((C, F), np.float32)
        xe[:n] = x[sel]
        xt_host = _round_tf32(np.ascontiguousarray(xe.T)).reshape(KC, 128, C)
        w1e, v1e, w2e = w1[e], v1[e], w2[e]
        w1t_host = _round_tf32(np.ascontiguousarray(
            w1e.reshape(KC, 128, MH, 128).transpose(2, 1, 0, 3)))
        v1t_host = _round_tf32(np.ascontiguousarray(
            v1e.reshape(KC, 128, MH, 128).transpose(2, 1, 0, 3)))
        w2t_host = _round_tf32(np.ascontiguousarray(w2e.T).reshape(MH, 128, F))
        in_maps.append({"xt": xt_host, "w1t": w1t_host, "v1t": v1t_host,
                        "w2t": w2t_host})

    res = run_bass_kernel_spmd(nc, in_maps, core_ids=list(range(E)))

    out = np.zeros((T, F), np.float32)
    for e in range(E):
        sel = sels[e]
        n = len(sel)
        yte = res.results[e]["yt"].reshape(F, C)      # y^T
        ye = yte[:, :n].T                              # [n, F]
        out[sel] += cws[e][:, None] * ye
    return out.reshape(B, S, F)

